# revision 8
# baseline (speedup 1.0000x reference)
"""Grayscale + single-level 2x2 Haar DWT kernel for Trainium2 (8 cores, SPMD).

Full input x [16,3,1024,1024] f32 -> out [16,4,512,512] f32.
Batch-sharded: core i handles samples [2i, 2i+1].

Math per sample (BGR weights w=(0.114,0.587,0.299), all bands scaled by 0.5):
  gray = w0*x[0] + w1*x[1] + w2*x[2]
  a,b,c,d = gray[0::2,0::2], gray[0::2,1::2], gray[1::2,0::2], gray[1::2,1::2]
  cA,cH,cV,cD = 0.5*(a+b+c+d), 0.5*(a+b-c-d), 0.5*(a-b+c-d), 0.5*(a-b-c+d)

Layout: a "superband" is 512 consecutive input rows loaded as one 2MB
contiguous DMA into a [128, 4, 1024] tile (partition p holds rows
4p..4p+3). Two superbands cover one sample plane.

Engine split (per superband):
  ACT : c_i = x_i * (w_i/2) cast f32->fp16 (scale folded into the cast;
        3 ACTIVATE-Copy ops) - keeps the 1x-only STT off the DVE.
  DVE : t = c0+c1 ; g = t+c2 (fp16 dense TT, 2x_1p mode)
        srow/drow = g[:,0::2,:] +/- g[:,1::2,:] (fp16 dense, 2x;
        written into t's halves - t is dead after g)
        cA,cH,cV,cD = srow/drow[...,0::2] +/- [...,1::2] (f32 out, 1x)
  SYNC: all DMA descriptor gen (loads + stores) on the SP HWDGE ring;
        software-pipelined so loads for superband i+1 are issued before
        compute of superband i.

The LAST superband is processed as two half-pieces (channel-interleaved
1MB loads, per-half compute + immediate stores) to halve the serial
drain chain after the final load lands.
"""

import numpy as np

N_CORES = 8
B, C, H, W = 16, 3, 1024, 1024
HO, WO = H // 2, W // 2
SPC = B // N_CORES  # samples per core
SB = 2              # superbands per sample plane (512 input rows each)

W_BGR = (0.114, 0.587, 0.299)

_compiled = None


def _build():
    from concourse import bacc, mybir
    from concourse.tile import TileContext

    f32 = mybir.dt.float32
    f16 = mybir.dt.float16
    add = mybir.AluOpType.add
    sub = mybir.AluOpType.subtract

    nc = bacc.Bacc("TRN2", target_bir_lowering=False, debug=False)
    # same bytes as [SPC, C, H, W] f32, pre-shaped for superband DMA
    x = nc.declare_dram_parameter("x", [SPC, C, SB, 128, 4, W], f32, isOutput=False)
    out = nc.declare_dram_parameter(
        "out", [SPC, 4, SB, 128, 2, WO], f32, isOutput=True
    )

    chunks = [(s, sb) for s in range(SPC) for sb in range(SB)]
    last = len(chunks) - 1

    with TileContext(nc) as tc:
        with (
            tc.tile_pool(name="in_pool", bufs=2) as in_pool,
            tc.tile_pool(name="sc_pool", bufs=2) as sc_pool,
            tc.tile_pool(name="mid_pool", bufs=2) as mid_pool,
            tc.tile_pool(name="out_pool", bufs=2) as out_pool,
        ):
            in_tiles = {}

            def issue_loads(i):
                s, sb = chunks[i]
                ts = []
                for ch in range(C):
                    t = in_pool.tile(
                        [128, 4, W], f32, tag=f"in{ch}", name=f"in{ch}_{i}"
                    )
                    ts.append(t)
                if i == last:
                    # channel-interleaved half loads so the final pieces
                    # complete (all 3 channels) as early as possible
                    for h in range(2):
                        for ch in range(C):
                            nc.sync.dma_start(
                                out=ts[ch][:, 2 * h : 2 * h + 2, :],
                                in_=x[s, ch, sb, :, 2 * h : 2 * h + 2, :],
                            )
                else:
                    for ch in range(C):
                        nc.sync.dma_start(out=ts[ch][:, :, :], in_=x[s, ch, sb])
                in_tiles[i] = ts

            def compute_and_store(i):
                s, sb = chunks[i]
                ch_t = in_tiles.pop(i)
                sc = [
                    sc_pool.tile(
                        [128, 4, W], f16, tag=f"sc{ch}", name=f"sc{ch}_{i}"
                    )
                    for ch in range(C)
                ]
                t = mid_pool.tile([128, 4, W], f16, tag="t")
                halves = (0, 1) if i == last else (slice(None),)

                def run(hsl, ge_sl, go_sl, s_sl, d_sl, o_sl):
                    # hsl: middle-dim slice of the 4-row-quad tiles
                    # ge/go: even/odd input-row slices of g within the quad
                    # s/d: stage-1 dest rows inside t; o: out middle slice
                    for ch in range(C):
                        nc.scalar.mul(
                            sc[ch][:, hsl, :], ch_t[ch][:, hsl, :], W_BGR[ch] * 0.5
                        )
                    nc.vector.tensor_tensor(
                        t[:, hsl, :], sc[0][:, hsl, :], sc[1][:, hsl, :], add
                    )
                    g = sc[0]  # dead after the first TT; reuse its slot
                    nc.vector.tensor_tensor(
                        g[:, hsl, :], t[:, hsl, :], sc[2][:, hsl, :], add
                    )
                    ge, go = g[:, ge_sl, :], g[:, go_sl, :]
                    srow, drow = t[:, s_sl, :], t[:, d_sl, :]
                    nc.vector.tensor_tensor(srow, ge, go, add)
                    nc.vector.tensor_tensor(drow, ge, go, sub)
                    for band, (src, op) in enumerate(
                        ((srow, add), (drow, add), (srow, sub), (drow, sub))
                    ):
                        # band order: cA, cH, cV, cD
                        o = out_pool.tile(
                            [128, 2, WO], f32, tag=f"o{band}", name=f"o{band}_{i}"
                        )
                        osl = o[:, 0 : len(range(*o_sl.indices(2))), :]
                        nc.vector.tensor_tensor(
                            osl, src[:, :, 0:W:2], src[:, :, 1:W:2], op
                        )
                        nc.sync.dma_start(
                            out=out[s, band, sb, :, o_sl, :], in_=osl
                        )

                if i == last:
                    run(
                        slice(0, 2), slice(0, 1), slice(1, 2),
                        slice(0, 1), slice(1, 2), slice(0, 1),
                    )
                    run(
                        slice(2, 4), slice(2, 3), slice(3, 4),
                        slice(2, 3), slice(3, 4), slice(1, 2),
                    )
                else:
                    run(
                        slice(0, 4), slice(0, 4, 2), slice(1, 4, 2),
                        slice(0, 2), slice(2, 4), slice(0, 2),
                    )

            for i in range(len(chunks)):
                issue_loads(i)
                if i >= 1:
                    compute_and_store(i - 1)
            compute_and_store(last)
    nc.finalize()
    return nc


def kernel(x: np.ndarray) -> np.ndarray:
    global _compiled
    from concourse.bass_utils import run_bass_kernel_spmd

    if _compiled is None:
        _compiled = _build()
    nc = _compiled

    x = np.ascontiguousarray(x, dtype=np.float32)
    in_maps = [{"x": x[i * SPC : (i + 1) * SPC]} for i in range(N_CORES)]
    res = run_bass_kernel_spmd(nc, in_maps, list(range(N_CORES))).results
    out = np.concatenate(
        [r["out"].reshape(SPC, 4, HO, WO) for r in res], axis=0
    )
    return out
